# revision 69
# baseline (speedup 1.0000x reference)
"""Multi-head transposed (channel) attention kernel for Trainium2.

Reference computation (per batch b, head h, c=32 channels, n=65536 spatial):
    q,k,v = split(qkv)                       # each [32, n] per (b,h)
    qh = q / max(||q||_row, 1e-12)           # L2 normalize over n
    kh = k / max(||k||_row, 1e-12)
    S = (qh @ kh.T) * temperature[h]         # [32, 32]
    A = softmax(S, axis=-1)
    out = A @ v                              # [32, n]

Sharding: 24 (b,h) pairs over 8 cores = 3 pairs/core, stacked on 96
partitions.  On the host, q and k are L2-normalized, temperature is folded
into q's rows, both are scaled by 64 and cast to fp8 e4m3 (values ~N(0,.25)
sit in e4m3's sweet spot); v is cast to fp16.  The output is produced in
fp16 and upcast on the host.

qk is pre-transposed on the host into the exact SBUF tile layout
[chunk, 128 (spatial), sub, 192 (q|k channels)], so pass-1 loads are fully
contiguous plain DMAs at HBM line rate.

Per core:
  pass 1: stream qk tiles; per 128-spatial sub ONE fp8 matmul accumulates
          S^T = k.T-chunk.T @ q.T-chunk into PSUM (96 moving cols, FWL
          weight loads hidden) - S lands already transposed for pass 2.
  chain:  exp fuses the 1/4096 descale and writes block-diagonal fp16
          attn^T directly from PSUM; softmax denominators via a
          ones-vector matmul, transposed back onto partitions.
  pass 2: out = attn^T-block-diag @ v in fp16 N=512 matmuls; PSUM->SBUF
          copies (with 1/rowsum scale) alternate between DVE and ACT.
          All of v is SBUF-resident; its 16 loads are posted on the sync
          ring right after the last qk load so DMA never idles across the
          pass-1 -> pass-2 transition.
"""

import ml_dtypes
import numpy as np

import concourse.bass as bass
import concourse.tile as tile
from concourse import bacc, mybir
from concourse.bass_utils import run_bass_kernel_spmd

F32 = mybir.dt.float32
F16 = mybir.dt.float16
F8 = mybir.dt.float8e4

B = 4
HD = 6
CH = 32          # channels per head
HW = 65536       # spatial size (256*256)
P = 96           # partition stack: 3 pairs * 32 channels
P2 = 192         # q-stack + k-stack channels
N_CORES = 8
PAIRS_PER_CORE = 3

QSCALE = 64.0    # host-side scale on normalized q and k before fp8 cast
DESCALE = 1.0 / (QSCALE * QSCALE)

FT = 4096        # pass-1 transpose-DMA chunk (spatial)
NCH1 = HW // FT  # 16
SUB = 128
NSUB = FT // SUB  # 32
NF = 512         # matmul free size (one PSUM bank)
# out-store chunks: thin leading chunks so real store traffic starts right
# after the softmax chain (concurrent writes are what keep the DMA engines'
# read streams at full rate), then 4096-col chunks
OCH = [4096] * 16
OOFF = [sum(OCH[:i]) for i in range(len(OCH))]
# v-load chunks: 6KB/partition descriptors, matching the qk loads (6KB
# reads stream at ~26GB/s/engine; 8KB and 16KB reads measured ~15GB/s in
# load-only phases).  65536 = 21*3072 + 1024; both are multiples of 512.
VCH = [3072] * 21 + [1024]
VOFF = [sum(VCH[:i]) for i in range(len(VCH))]


def build_nc():
    nc = bacc.Bacc("TRN2", target_bir_lowering=False, debug=False,
                   num_devices=N_CORES)
    qk_d = nc.dram_tensor("qk", [NCH1, SUB, NSUB, P2], F8,
                          kind="ExternalInput").ap()
    # v and out are chunk-major so every DMA walks near-contiguous HBM
    # (sequential descriptors stream ~26GB/s per engine; the strided
    # [96, HW] walk measured ~40% slower)
    v_d = [nc.dram_tensor(f"v{t}", [P, w], F16, kind="ExternalInput").ap()
           for t, w in enumerate(VCH)]
    o_d = [nc.dram_tensor(f"o{t}", [P, w], F16, kind="ExternalOutput").ap()
           for t, w in enumerate(OCH)]
    scr_d = nc.dram_tensor("scr", [32, P, 256], F16, kind="Internal").ap()

    with tile.TileContext(nc) as tc:
        _body(nc, tc, qk_d, v_d, o_d, scr_d)
    nc.compile()
    return nc


def _body(nc, tc, qk_d, v_d, o_d, scr_d):
    Exp = mybir.ActivationFunctionType.Exp
    Copy = mybir.ActivationFunctionType.Copy

    with tc.tile_pool(name="persist", bufs=1) as pp:
        # warm both ACT tables off the critical path: Copy first, Exp last
        # so the chain's exps find the Exp table loaded (pass 2's first ACT
        # copy then re-loads Copy, overlapped with DVE work)
        warm = pp.tile([1, 1], F32)
        nc.gpsimd.memset(warm[:, :], 1.0)
        nc.scalar.activation(out=warm[:, :], in_=warm[:, :], func=Copy)
        nc.scalar.activation(out=warm[:, :], in_=warm[:, :], func=Exp)
        # scratch operand for the PE keep-warm fillers
        wsc = pp.tile([SUB, NF], F8)
        nc.gpsimd.memset(wsc[:, :], 0.0)

        E_sb = pp.tile([P, P], F16)
        nc.gpsimd.memset(E_sb[:, :], 0.0)
        ones96 = pp.tile([P, 1], F16)
        nc.gpsimd.memset(ones96[:, :], 1.0)
        ident1 = pp.tile([1, 1], F32)
        nc.gpsimd.memset(ident1[:, :], 1.0)
        rs_sb = pp.tile([1, P], F32)
        rinv = pp.tile([P, 1], F32)

        # one PSUM bank accumulates S^T = kT.T @ qT over all 512 subs
        psS_cm = tc.tile_pool(name="psS", bufs=1, space="PSUM")
        psS_p = psS_cm.__enter__()
        acc = psS_p.tile([P, P], F32)

        # v is fully SBUF-resident; allocate all tiles up front so the
        # loads can be posted with no buffer-recycle waits
        iov = tc.tile_pool(name="iov", bufs=1)
        iov_p = iov.__enter__()
        v_tiles = [iov_p.tile([P, w], F16, tag=f"v{t}", name=f"v{t}")
                   for t, w in enumerate(VCH)]

        # ---------------- pass 1: S^T ----------------
        with tc.tile_pool(name="io1", bufs=6) as io1:
            for t in range(NCH1):
                qkT = io1.tile([SUB, NSUB, P2], F8, tag="qkT")
                # alternate rings: two queues per DMA engine overlap
                # transfer latency (single-queue 8KB streams measured ~15
                # GB/s/engine vs ~26 when an engine serves two queues)
                nc.sync.dma_start(out=qkT[:, :, :], in_=qk_d[t])
                for s in range(NSUB):
                    first = (t == 0 and s == 0)
                    last = (t == NCH1 - 1 and s == NSUB - 1)
                    nc.tensor.matmul(
                        acc[:, :],
                        lhsT=qkT[:, s, P:P2],
                        rhs=qkT[:, s, 0:P],
                        start=first, stop=last, skip_group_check=True)

        # post all v loads now: they queue on the sync ring behind the
        # final qk loads, keeping the DMA engines saturated through the
        # softmax chain and into pass 2
        for t in range(len(VCH)):
            nc.sync.dma_start(out=v_tiles[t][:, :], in_=v_d[t][:, :])

        # PE keep-warm filler: occupies the PE during the softmax chain so
        # the HAM clock gate stays at 8/8 into pass 2 (results unused)
        wacc = psS_p.tile([P, NF], F32, tag="warm")
        for w in range(12):
            nc.tensor.matmul(
                wacc[:, :], lhsT=wsc[:, 0:P], rhs=wsc[:, :],
                start=(w == 0), stop=(w == 11), skip_group_check=True)

        # ---------------- softmax chain ----------------
        with tc.tile_pool(name="psC", bufs=1, space="PSUM") as psC:
            # block-diagonal unnormalized attn^T in fp16, straight from PSUM
            for j in range(PAIRS_PER_CORE):
                blk = slice(CH * j, CH * (j + 1))
                nc.scalar.activation(out=E_sb[blk, blk], in_=acc[blk, blk],
                                     func=Exp, scale=DESCALE)
            # softmax denominators: column sums of E via ones-matmul,
            # transposed back onto partitions
            rs_ps = psC.tile([1, P], F32, tag="rs")
            nc.tensor.matmul(rs_ps[:, :], lhsT=ones96[:, :], rhs=E_sb[:, :],
                             start=True, stop=True)
            nc.vector.tensor_copy(out=rs_sb[:, :], in_=rs_ps[:, :])
            ri_ps = psC.tile([P, 1], F32, tag="ri")
            nc.tensor.transpose(ri_ps[:, :], rs_sb[:, :], ident1[:, :])
            nc.vector.reciprocal(out=rinv[:, :], in_=ri_ps[:, :])

        # release the accumulator bank so pass 2 can use 8 PSUM banks
        psS_cm.__exit__(None, None, None)

        # ---------------- pass 2: out = attn @ v ----------------
        with (
            tc.tile_pool(name="ioo", bufs=4) as ioo,
            tc.tile_pool(name="psO", bufs=3, space="PSUM") as psOp,
            tc.tile_pool(name="psW", bufs=1, space="PSUM") as psWp,
        ):
            mult = mybir.AluOpType.mult
            wacc2 = psWp.tile([P, NF], F32)
            nw = 0
            cu = 0
            for t, cols in enumerate(OCH):
                # keep-warm fillers: absorb the PE idle while waiting for v
                # chunks / psum banks, so the HAM clock gate stays at 8/8
                # (in-order PE queue: at most ~0.6us of real-work delay)
                for _ in range(1 + cols // 2048):
                    nc.tensor.matmul(
                        wacc2[:, :], lhsT=wsc[:, 0:P], rhs=wsc[:, :],
                        start=(nw == 0), stop=False, skip_group_check=True)
                    nw += 1
                on = ioo.tile([P, cols], F16, tag="on")
                for h in range(cols // (2 * NF)):
                    # two 512-col matmuls fill a 2-bank psum tile; one
                    # 1024-col copy drains it (fewer, larger engine ops)
                    o_ps = psOp.tile([P, 2 * NF], F32, tag="o")
                    for m in range(2):
                        base = OOFF[t] + (2 * h + m) * NF
                        ti = next(i for i in reversed(range(len(VCH)))
                                  if VOFF[i] <= base)
                        msl = slice(base - VOFF[ti], base - VOFF[ti] + NF)
                        nc.tensor.matmul(o_ps[:, m * NF:(m + 1) * NF],
                                         lhsT=E_sb[:, :],
                                         rhs=v_tiles[ti][:, msl],
                                         start=True, stop=True)
                    csl = slice(2 * h * NF, 2 * (h + 1) * NF)
                    if cu % 2 == 0:
                        nc.vector.tensor_scalar(
                            out=on[:, csl], in0=o_ps[:, :],
                            scalar1=rinv[:, :], scalar2=None, op0=mult)
                    else:
                        nc.scalar.activation(out=on[:, csl], in_=o_ps[:, :],
                                             func=Copy, scale=rinv[:, :])
                    cu += 1
                # posted from the (idle) gpsimd sequencer: the ACT queue is
                # busy with copies and would delay the store posts
                nc.gpsimd.dma_start(out=o_d[t], in_=on[:, :])
            nc.tensor.matmul(
                wacc2[:, :], lhsT=wsc[:, 0:P], rhs=wsc[:, :],
                start=False, stop=True, skip_group_check=True)

        iov.__exit__(None, None, None)


_NC_CACHE = {}


def _get_nc():
    if "nc" not in _NC_CACHE:
        _NC_CACHE["nc"] = build_nc()
    return _NC_CACHE["nc"]


def _shard_inputs(qkv, temperature):
    qkv = np.asarray(qkv)
    temp = np.asarray(temperature, dtype=np.float32).reshape(-1)
    C = HD * CH
    q = qkv[:, 0 * C:1 * C].reshape(B, HD, CH, HW)
    k = qkv[:, 1 * C:2 * C].reshape(B, HD, CH, HW)
    v = qkv[:, 2 * C:3 * C].reshape(B, HD, CH, HW)

    # fold L2 normalization, temperature, and the fp8 range scale into the
    # host-side quantization of q and k
    qn = np.maximum(np.sqrt(np.einsum('bhcn,bhcn->bhc', q, q)), 1e-12)
    kn = np.maximum(np.sqrt(np.einsum('bhcn,bhcn->bhc', k, k)), 1e-12)
    qs = (QSCALE * temp[None, :, None] / qn)[..., None]
    ks = (QSCALE / kn)[..., None]
    q8 = (q * qs).astype(ml_dtypes.float8_e4m3)
    k8 = (k * ks).astype(ml_dtypes.float8_e4m3)

    in_maps = []
    for core in range(N_CORES):
        pairs = [divmod(p, HD) for p in
                 range(core * PAIRS_PER_CORE, (core + 1) * PAIRS_PER_CORE)]
        qs_ = np.concatenate([q8[b_, h_] for b_, h_ in pairs], axis=0)
        ks_ = np.concatenate([k8[b_, h_] for b_, h_ in pairs], axis=0)
        qks = np.concatenate([qs_, ks_], axis=0)
        # pre-transpose to the SBUF tile layout [chunk, p, sub, ch]
        qks = np.ascontiguousarray(
            qks.reshape(P2, NCH1, NSUB, SUB).transpose(1, 3, 2, 0))
        vs = np.concatenate([v[b_, h_] for b_, h_ in pairs],
                            axis=0).astype(np.float16)
        # one contiguous HBM block per v chunk
        im = {"qk": qks}
        for t, w in enumerate(VCH):
            im[f"v{t}"] = np.ascontiguousarray(vs[:, VOFF[t]:VOFF[t] + w])
        in_maps.append(im)
    return in_maps


def _gather_output(results):
    out = np.empty((B, HD, CH, HW), dtype=np.float32)
    for core in range(N_CORES):
        o = np.concatenate(
            [results[core][f"o{t}"] for t in range(len(OCH))], axis=1)
        for j in range(PAIRS_PER_CORE):
            b_, h_ = divmod(core * PAIRS_PER_CORE + j, HD)
            out[b_, h_] = o[CH * j:CH * (j + 1)].astype(np.float32)
    return out.reshape(B, HD * CH, 256, 256)


def kernel(qkv, temperature):
    in_maps = _shard_inputs(qkv, temperature)
    nc = _get_nc()
    res = run_bass_kernel_spmd(nc, in_maps, list(range(N_CORES)))
    return _gather_output(res.results)


if __name__ == "__main__":
    rng = np.random.default_rng(0)
    qkv = rng.standard_normal((B, 576, 256, 256), dtype=np.float32)
    temp = np.ones((HD, 1, 1), dtype=np.float32)
    out = kernel(qkv=qkv, temperature=temp)
    print("out", out.shape, out.dtype, float(np.abs(out).max()))


# revision 70
# speedup vs baseline: 1.0536x; 1.0536x over previous
"""Multi-head transposed (channel) attention kernel for Trainium2.

Reference computation (per batch b, head h, c=32 channels, n=65536 spatial):
    q,k,v = split(qkv)                       # each [32, n] per (b,h)
    qh = q / max(||q||_row, 1e-12)           # L2 normalize over n
    kh = k / max(||k||_row, 1e-12)
    S = (qh @ kh.T) * temperature[h]         # [32, 32]
    A = softmax(S, axis=-1)
    out = A @ v                              # [32, n]

Sharding: 24 (b,h) pairs over 8 cores = 3 pairs/core, stacked on 96
partitions.  On the host, q and k are L2-normalized, temperature is folded
into q's rows, both are scaled by 64 and cast to fp8 e4m3 (values ~N(0,.25)
sit in e4m3's sweet spot); v is cast to fp16.  The output is produced in
fp16 and upcast on the host.

qk is pre-transposed on the host into the exact SBUF tile layout
[chunk, 128 (spatial), sub, 192 (q|k channels)], so pass-1 loads are fully
contiguous plain DMAs at HBM line rate.

Per core:
  pass 1: stream qk tiles; per 128-spatial sub ONE fp8 matmul accumulates
          S^T = k.T-chunk.T @ q.T-chunk into PSUM (96 moving cols, FWL
          weight loads hidden) - S lands already transposed for pass 2.
  chain:  exp fuses the 1/4096 descale and writes block-diagonal fp16
          attn^T directly from PSUM; softmax denominators via a
          ones-vector matmul, transposed back onto partitions.
  pass 2: out = attn^T-block-diag @ v in fp16 N=512 matmuls; PSUM->SBUF
          copies (with 1/rowsum scale) alternate between DVE and ACT.
          All of v is SBUF-resident; its 16 loads are posted on the sync
          ring right after the last qk load so DMA never idles across the
          pass-1 -> pass-2 transition.
"""

import ml_dtypes
import numpy as np

import concourse.bass as bass
import concourse.tile as tile
from concourse import bacc, mybir
from concourse.bass_utils import run_bass_kernel_spmd

F32 = mybir.dt.float32
F16 = mybir.dt.float16
F8 = mybir.dt.float8e4

B = 4
HD = 6
CH = 32          # channels per head
HW = 65536       # spatial size (256*256)
P = 96           # partition stack: 3 pairs * 32 channels
P2 = 192         # q-stack + k-stack channels
N_CORES = 8
PAIRS_PER_CORE = 3

QSCALE = 64.0    # host-side scale on normalized q and k before fp8 cast
DESCALE = 1.0 / (QSCALE * QSCALE)

FT = 4096        # pass-1 transpose-DMA chunk (spatial)
NCH1 = HW // FT  # 16
SUB = 128
NSUB = FT // SUB  # 32
NF = 512         # matmul free size (one PSUM bank)
# out-store chunks: thin leading chunks so real store traffic starts right
# after the softmax chain (concurrent writes are what keep the DMA engines'
# read streams at full rate), then 4096-col chunks
OCH = [4096] * 16
OOFF = [sum(OCH[:i]) for i in range(len(OCH))]
# v-load chunks: 6KB/partition descriptors, matching the qk loads (6KB
# reads stream at ~26GB/s/engine; 8KB and 16KB reads measured ~15GB/s in
# load-only phases).  65536 = 21*3072 + 1024; both are multiples of 512.
VCH = [3072] * 21 + [1024]
VOFF = [sum(VCH[:i]) for i in range(len(VCH))]


def build_nc():
    nc = bacc.Bacc("TRN2", target_bir_lowering=False, debug=False,
                   num_devices=N_CORES)
    qk_d = nc.dram_tensor("qk", [NCH1, SUB, NSUB, P2], F8,
                          kind="ExternalInput").ap()
    # v and out are chunk-major so every DMA walks near-contiguous HBM
    # (sequential descriptors stream ~26GB/s per engine; the strided
    # [96, HW] walk measured ~40% slower)
    v_d = [nc.dram_tensor(f"v{t}", [P, w], F16, kind="ExternalInput").ap()
           for t, w in enumerate(VCH)]
    o_d = [nc.dram_tensor(f"o{t}", [P, w], F16, kind="ExternalOutput").ap()
           for t, w in enumerate(OCH)]
    scr_d = nc.dram_tensor("scr", [32, P, 256], F16, kind="Internal").ap()

    with tile.TileContext(nc) as tc:
        _body(nc, tc, qk_d, v_d, o_d, scr_d)
    nc.compile()
    return nc


def _body(nc, tc, qk_d, v_d, o_d, scr_d):
    Exp = mybir.ActivationFunctionType.Exp
    Copy = mybir.ActivationFunctionType.Copy

    with tc.tile_pool(name="persist", bufs=1) as pp:
        # warm both ACT tables off the critical path: Copy first, Exp last
        # so the chain's exps find the Exp table loaded (pass 2's first ACT
        # copy then re-loads Copy, overlapped with DVE work)
        warm = pp.tile([1, 1], F32)
        nc.gpsimd.memset(warm[:, :], 1.0)
        nc.scalar.activation(out=warm[:, :], in_=warm[:, :], func=Copy)
        nc.scalar.activation(out=warm[:, :], in_=warm[:, :], func=Exp)
        # scratch operand for the PE keep-warm fillers
        wsc = pp.tile([SUB, NF], F8)
        nc.gpsimd.memset(wsc[:, :], 0.0)

        E_sb = pp.tile([P, P], F16)
        nc.gpsimd.memset(E_sb[:, :], 0.0)
        ones96 = pp.tile([P, 1], F16)
        nc.gpsimd.memset(ones96[:, :], 1.0)
        ident1 = pp.tile([1, 1], F32)
        nc.gpsimd.memset(ident1[:, :], 1.0)
        rs_sb = pp.tile([1, P], F32)
        rinv = pp.tile([P, 1], F32)

        # one PSUM bank accumulates S^T = kT.T @ qT over all 512 subs
        psS_cm = tc.tile_pool(name="psS", bufs=1, space="PSUM")
        psS_p = psS_cm.__enter__()
        acc = psS_p.tile([P, P], F32)

        # v is fully SBUF-resident; allocate all tiles up front so the
        # loads can be posted with no buffer-recycle waits
        iov = tc.tile_pool(name="iov", bufs=1)
        iov_p = iov.__enter__()
        v_tiles = [iov_p.tile([P, w], F16, tag=f"v{t}", name=f"v{t}")
                   for t, w in enumerate(VCH)]

        # ---------------- pass 1: S^T ----------------
        with tc.tile_pool(name="io1", bufs=6) as io1:
            for t in range(NCH1):
                qkT = io1.tile([SUB, NSUB, P2], F8, tag="qkT")
                # alternate rings: two queues per DMA engine overlap
                # transfer latency (single-queue 8KB streams measured ~15
                # GB/s/engine vs ~26 when an engine serves two queues)
                nc.sync.dma_start(out=qkT[:, :, :], in_=qk_d[t])
                for s in range(NSUB):
                    first = (t == 0 and s == 0)
                    last = (t == NCH1 - 1 and s == NSUB - 1)
                    nc.tensor.matmul(
                        acc[:, :],
                        lhsT=qkT[:, s, P:P2],
                        rhs=qkT[:, s, 0:P],
                        start=first, stop=last, skip_group_check=True)

        # post all v loads now: they queue on the sync ring behind the
        # final qk loads, keeping the DMA engines saturated through the
        # softmax chain and into pass 2
        for t in range(len(VCH)):
            nc.sync.dma_start(out=v_tiles[t][:, :], in_=v_d[t][:, :])

        # PE keep-warm filler: occupies the PE during the softmax chain so
        # the HAM clock gate stays at 8/8 into pass 2 (results unused)
        wacc = psS_p.tile([P, NF], F32, tag="warm")
        for w in range(12):
            nc.tensor.matmul(
                wacc[:, :], lhsT=wsc[:, 0:P], rhs=wsc[:, :],
                start=(w == 0), stop=(w == 11), skip_group_check=True)

        # ---------------- softmax chain ----------------
        with tc.tile_pool(name="psC", bufs=1, space="PSUM") as psC:
            # block-diagonal unnormalized attn^T in fp16, straight from PSUM
            for j in range(PAIRS_PER_CORE):
                blk = slice(CH * j, CH * (j + 1))
                nc.scalar.activation(out=E_sb[blk, blk], in_=acc[blk, blk],
                                     func=Exp, scale=DESCALE)
            # softmax denominators: column sums of E via ones-matmul,
            # transposed back onto partitions
            rs_ps = psC.tile([1, P], F32, tag="rs")
            nc.tensor.matmul(rs_ps[:, :], lhsT=ones96[:, :], rhs=E_sb[:, :],
                             start=True, stop=True)
            nc.vector.tensor_copy(out=rs_sb[:, :], in_=rs_ps[:, :])
            ri_ps = psC.tile([P, 1], F32, tag="ri")
            nc.tensor.transpose(ri_ps[:, :], rs_sb[:, :], ident1[:, :])
            nc.vector.reciprocal(out=rinv[:, :], in_=ri_ps[:, :])

        # release the accumulator bank so pass 2 can use 8 PSUM banks
        psS_cm.__exit__(None, None, None)

        # ---------------- pass 2: out = attn @ v ----------------
        # junk stores paced by early v-chunk arrivals: keep a write stream
        # alive on the DMA engines from the moment v starts loading (reads
        # only run at full rate with writes in flight, and faster group
        # completions also unthrottle the sem-paced dma_start posts).
        # Ring order keeps each junk ahead of real stores that would wait.
        jmap = {0: [0, 1], 1: [2], 2: [3], 3: [4], 4: [5]}
        with (
            tc.tile_pool(name="ioo", bufs=4) as ioo,
            tc.tile_pool(name="psO", bufs=3, space="PSUM") as psOp,
            tc.tile_pool(name="psW", bufs=1, space="PSUM") as psWp,
        ):
            mult = mybir.AluOpType.mult
            wacc2 = psWp.tile([P, NF], F32)
            nw = 0
            cu = 0
            for t, cols in enumerate(OCH):
                for vj in jmap.get(t, []):
                    nc.gpsimd.dma_start(out=scr_d[vj],
                                        in_=v_tiles[vj][:, 0:256])
                # keep-warm fillers: absorb the PE idle while waiting for v
                # chunks / psum banks, so the HAM clock gate stays at 8/8
                # (in-order PE queue: at most ~0.6us of real-work delay)
                for _ in range(1 + cols // 2048):
                    nc.tensor.matmul(
                        wacc2[:, :], lhsT=wsc[:, 0:P], rhs=wsc[:, :],
                        start=(nw == 0), stop=False, skip_group_check=True)
                    nw += 1
                on = ioo.tile([P, cols], F16, tag="on")
                for h in range(cols // (2 * NF)):
                    # two 512-col matmuls fill a 2-bank psum tile; one
                    # 1024-col copy drains it (fewer, larger engine ops)
                    o_ps = psOp.tile([P, 2 * NF], F32, tag="o")
                    for m in range(2):
                        base = OOFF[t] + (2 * h + m) * NF
                        ti = next(i for i in reversed(range(len(VCH)))
                                  if VOFF[i] <= base)
                        msl = slice(base - VOFF[ti], base - VOFF[ti] + NF)
                        nc.tensor.matmul(o_ps[:, m * NF:(m + 1) * NF],
                                         lhsT=E_sb[:, :],
                                         rhs=v_tiles[ti][:, msl],
                                         start=True, stop=True)
                    csl = slice(2 * h * NF, 2 * (h + 1) * NF)
                    if cu % 2 == 0:
                        nc.vector.tensor_scalar(
                            out=on[:, csl], in0=o_ps[:, :],
                            scalar1=rinv[:, :], scalar2=None, op0=mult)
                    else:
                        nc.scalar.activation(out=on[:, csl], in_=o_ps[:, :],
                                             func=Copy, scale=rinv[:, :])
                    cu += 1
                # posted from the (idle) gpsimd sequencer: the ACT queue is
                # busy with copies and would delay the store posts
                nc.gpsimd.dma_start(out=o_d[t], in_=on[:, :])
            nc.tensor.matmul(
                wacc2[:, :], lhsT=wsc[:, 0:P], rhs=wsc[:, :],
                start=False, stop=True, skip_group_check=True)

        iov.__exit__(None, None, None)


_NC_CACHE = {}


def _get_nc():
    if "nc" not in _NC_CACHE:
        _NC_CACHE["nc"] = build_nc()
    return _NC_CACHE["nc"]


def _shard_inputs(qkv, temperature):
    qkv = np.asarray(qkv)
    temp = np.asarray(temperature, dtype=np.float32).reshape(-1)
    C = HD * CH
    q = qkv[:, 0 * C:1 * C].reshape(B, HD, CH, HW)
    k = qkv[:, 1 * C:2 * C].reshape(B, HD, CH, HW)
    v = qkv[:, 2 * C:3 * C].reshape(B, HD, CH, HW)

    # fold L2 normalization, temperature, and the fp8 range scale into the
    # host-side quantization of q and k
    qn = np.maximum(np.sqrt(np.einsum('bhcn,bhcn->bhc', q, q)), 1e-12)
    kn = np.maximum(np.sqrt(np.einsum('bhcn,bhcn->bhc', k, k)), 1e-12)
    qs = (QSCALE * temp[None, :, None] / qn)[..., None]
    ks = (QSCALE / kn)[..., None]
    q8 = (q * qs).astype(ml_dtypes.float8_e4m3)
    k8 = (k * ks).astype(ml_dtypes.float8_e4m3)

    in_maps = []
    for core in range(N_CORES):
        pairs = [divmod(p, HD) for p in
                 range(core * PAIRS_PER_CORE, (core + 1) * PAIRS_PER_CORE)]
        qs_ = np.concatenate([q8[b_, h_] for b_, h_ in pairs], axis=0)
        ks_ = np.concatenate([k8[b_, h_] for b_, h_ in pairs], axis=0)
        qks = np.concatenate([qs_, ks_], axis=0)
        # pre-transpose to the SBUF tile layout [chunk, p, sub, ch]
        qks = np.ascontiguousarray(
            qks.reshape(P2, NCH1, NSUB, SUB).transpose(1, 3, 2, 0))
        vs = np.concatenate([v[b_, h_] for b_, h_ in pairs],
                            axis=0).astype(np.float16)
        # one contiguous HBM block per v chunk
        im = {"qk": qks}
        for t, w in enumerate(VCH):
            im[f"v{t}"] = np.ascontiguousarray(vs[:, VOFF[t]:VOFF[t] + w])
        in_maps.append(im)
    return in_maps


def _gather_output(results):
    out = np.empty((B, HD, CH, HW), dtype=np.float32)
    for core in range(N_CORES):
        o = np.concatenate(
            [results[core][f"o{t}"] for t in range(len(OCH))], axis=1)
        for j in range(PAIRS_PER_CORE):
            b_, h_ = divmod(core * PAIRS_PER_CORE + j, HD)
            out[b_, h_] = o[CH * j:CH * (j + 1)].astype(np.float32)
    return out.reshape(B, HD * CH, 256, 256)


def kernel(qkv, temperature):
    in_maps = _shard_inputs(qkv, temperature)
    nc = _get_nc()
    res = run_bass_kernel_spmd(nc, in_maps, list(range(N_CORES)))
    return _gather_output(res.results)


if __name__ == "__main__":
    rng = np.random.default_rng(0)
    qkv = rng.standard_normal((B, 576, 256, 256), dtype=np.float32)
    temp = np.ones((HD, 1, 1), dtype=np.float32)
    out = kernel(qkv=qkv, temperature=temp)
    print("out", out.shape, out.dtype, float(np.abs(out).max()))
